# revision 65
# baseline (speedup 1.0000x reference)
"""Trainium2 Bass kernel for nn_Attention_XL (B=2,T=2048,C=1024,S=2048,H=16).

Sharding: (batch, head) pairs across 8 cores — core c handles batch c//4 and
heads [(c%4)*4, (c%4)*4+4). QKV projection is column-sharded by head (no
redundant FLOPs); W_proj is column-sharded; per-core partial outputs are
summed on the host (the tensor-parallel unshard step).

Per-core device program (feature-on-partition layouts, fp32r matmuls). The
schedule is built around keeping the ACT engine's exp stream (the serial
floor, ~266us) saturated from ~10us on:
  - q is DMA'd t-chunk-major so qn(t0) exists after ~2.4MB of traffic; the
    first scores/exp chunk issues right after.
  - all remaining projection work (kn chunks, vn chunks, pair-1 qn/kn) is
    injected between attention l-chunks, each lump borrowing an sc-pool
    PSUM tile, paced to fit ACT's per-chunk slack.
  - attn@v accumulators (oA/oB) are double-buffered so t-chunk boundaries
    don't stall; normalize runs early in the next window. Head-odd v_aug
    columns are [ones,dims] so odd outputs land on partitions 64..127,
    letting one borrowed PSUM tile hold both heads' 1/sumexp broadcast and
    the output pair pack into [128,T] y tiles.
  - the output projection contracts K=128 (two heads per matmul) against
    the packed y tiles and drips through pair-1's windows.
"""
import sys

sys.path.insert(0, "/opt/trn_rl_repo")

import numpy as np
from ml_dtypes import bfloat16 as np_bf16
import concourse.bass as bass
import concourse.bacc as bacc
import concourse.mybir as mybir
import concourse.tile as tile
from concourse.bass_utils import run_bass_kernel_spmd

F32 = mybir.dt.float32
F32R = mybir.dt.float32r
BF16 = mybir.dt.bfloat16
AF = mybir.ActivationFunctionType
ADD = mybir.AluOpType.add

HD = 64          # head dim
HPC = 4          # heads per core
N_CORES = 8


def r(ap):
    return ap.bitcast(F32R)


def build_program(T, S, C, timing_mode=False):
    """Build + compile the per-core Bass program. Same program on all cores.

    timing_mode: big I/O tensors become Internal DRAM (no host transfer) so
    pipelined wall-clock isolates device exec; compute is unchanged."""
    L = S + T
    nL = L // 128           # key chunks
    nS = S // 128
    nT = T // 128
    nC = C // 128           # contraction chunks for qkv proj
    nTc = T // 512          # 512-wide t chunks
    R = 3 * HPC * HD        # rows of W' (768)
    scale = 1.0 / np.sqrt(HD)

    nc = bacc.Bacc("TRN2", target_bir_lowering=False, debug=False)

    ik = "Internal" if timing_mode else "ExternalInput"
    ok = "Internal" if timing_mode else "ExternalOutput"
    qT = nc.dram_tensor("qT", [C, T], BF16, kind=ik).ap()
    wqkvT = nc.dram_tensor("wqkvT", [C, R], BF16, kind=ik).ap()
    wp2 = nc.dram_tensor("wp2", [128, 2 * C], F32, kind=ik).ap()
    kxlT = nc.dram_tensor("kxlT", [HPC * HD, S], BF16, kind=ik).ap()
    posT = nc.dram_tensor("posT", [HPC * HD, S], BF16, kind=ik).ap()
    vxl = nc.dram_tensor("vxl", [S, HPC * HD], BF16, kind=ik).ap()
    n_ones = 192
    ones_in = nc.dram_tensor("ones", [128, n_ones], F32, kind=ik).ap()
    outT = nc.dram_tensor("outT", [C, T], F32, kind=ok).ap()
    if timing_mode:
        dummy = nc.dram_tensor("tm_in", [128, 128], F32,
                               kind="ExternalInput").ap()
        tiny = nc.dram_tensor("tm_out", [128, 128], F32,
                              kind="ExternalOutput").ap()

    with tile.TileContext(nc) as tc:
        import contextlib
        with contextlib.ExitStack() as ctx:
            persist = ctx.enter_context(tc.tile_pool(name="persist", bufs=1))
            vaugp = ctx.enter_context(tc.tile_pool(name="vaugp", bufs=1))
            att2 = ctx.enter_context(tc.tile_pool(name="att_sb", bufs=1))
            sc_ps = ctx.enter_context(
                tc.tile_pool(name="sc_ps", bufs=3, space="PSUM"))
            mm2_ps = ctx.enter_context(
                tc.tile_pool(name="mm2_ps", bufs=1, space="PSUM"))

            ones_sb = persist.tile([128, 64], F32, tag="ones_sb")
            sel_sb = persist.tile([128, 128], F32, tag="sel_sb")
            qnT = [persist.tile([128, T], BF16, tag=f"qnT{p}",
                    name=f"qnT{p}") for p in range(2)]
            kcatT = [persist.tile([128, L], BF16, tag=f"kcatT{p}",
                                  name=f"kcatT{p}") for p in range(2)]
            yTp = [persist.tile([128, T], F32, tag=f"yTp{p}", name=f"yTp{p}")
                   for p in range(2)]
            # v_aug is bf16 (matches et); head-even columns [dims(64),
            # ones(2)] -> attn@v out parts 0..65 (den at 64); head-odd
            # columns [ones(64), dims(64)] -> out parts 0..127 (den at
            # 0..63, dims at 64..127) so y packs into [128,T] tiles while
            # keeping the matmul base partition legal (0)
            VW = [66, 128, 66, 128]   # va width per head
            VD = [0, 64, 0, 64]       # dim column offset per head
            v_aug = [vaugp.tile([128, nL * VW[h]], BF16, tag=f"vaug{h}",
                                name=f"vaug{h}") for h in range(HPC)]
            var = [v_aug[h].rearrange("p (n w) -> p n w", w=VW[h])
                   for h in range(HPC)]

            def sc_tile():
                return sc_ps.tile([128, 1024], F32, tag="sc", name="sc")

            def oab_tile(which):
                return mm2_ps.tile([128, 512], F32, tag=which, name=which)

            # ---- building blocks -------------------------------------
            def qkv_m_t(wqm, m, t):
                # one (128-row m-tile, 512-col t-chunk) of the qkv proj,
                # accumulated in a borrowed sc tile
                ts = slice(t * 512, (t + 1) * 512)
                ps = sc_tile()
                for k in range(nC):
                    nc.tensor.matmul(
                        ps[:, 0:512], wqm[:, k, :], qt[:, k, ts],
                        start=(k == 0), stop=(k == nC - 1))
                dst = (qnT[m % 2][:, ts] if m < 2 else
                       kcatT[m % 2][:, S + t * 512:S + (t + 1) * 512])
                nc.vector.tensor_copy(dst, ps[:, 0:512])

            def vn_chunk(i):
                # vn for all 4 heads, t-rows i*128..+128, in [t, dim]
                # layout; one DVE cast to a staging tile releases the
                # borrowed sc bank fast, the idle Pool engine fans out
                ps = sc_tile()
                for k in range(nC):
                    nc.tensor.matmul(
                        ps[:, 0:256],
                        qt[:, k, i * 128:(i + 1) * 128],
                        wq45[:, k, :],
                        start=(k == 0), stop=(k == nC - 1))
                vsb = att2.tile([128, 256], BF16, tag="vnsb", bufs=2,
                                name="vnsb")
                nc.vector.tensor_copy(vsb[:], ps[:, 0:256])
                for h in range(HPC):
                    nc.gpsimd.tensor_copy(
                        var[h][:, nS + i, VD[h]:VD[h] + HD],
                        vsb[:, h * HD:(h + 1) * HD])

            def norm_front(p, oA, oB, ts):
                # Stage the sumexp rows to SBUF: A's sits on partition 64,
                # B's is replicated on parts 0..63 and staged from part 0
                sr = att2.tile([66, 512], F32, tag="srAB", bufs=1,
                               name="srAB")
                stg = att2.tile([128, 512], F32, tag="brec", bufs=1,
                                name="brec_stage")
                nc.vector.tensor_copy(r(sr[64:65, :]), oA[64:65, :])
                nc.vector.tensor_copy(r(stg[0:1, :]), oB[0:1, :])
                return (p, oA, oB, ts, sr, stg)

            def norm_back(st):
                # PE broadcast of both heads' sumexp into one borrowed PSUM
                # tile, one DVE reciprocal to SBUF, then PSUMxSBUF scales;
                # emitted a couple of chunks into the NEXT window so the bc
                # matmul never head-blocks PE
                p, oA, oB, ts, sr, stg = st
                bcf = sc_tile()
                nc.tensor.matmul(
                    bcf[0:64, 0:512], r(ones_sb[64:65, 0:64]),
                    r(sr[64:65, :]),
                    start=True, stop=True, tile_position=(64, 0))
                nc.tensor.matmul(
                    bcf[0:128, 512:1024], r(sel_sb[0:1, 0:128]),
                    r(stg[0:1, :]),
                    start=True, stop=True, tile_position=(0, 0))
                brec = att2.tile([128, 512], F32, tag="brec", bufs=1,
                                 name="brec")
                nc.vector.reciprocal(brec[0:64, :], bcf[0:64, 0:512])
                nc.vector.reciprocal(brec[64:128, :], bcf[64:128, 512:1024])
                nc.vector.tensor_mul(
                    r(yTp[p][0:64, ts]), oA[0:64, :], brec[0:64, :])
                nc.vector.tensor_mul(
                    r(yTp[p][64:128, ts]), oB[64:128, :], brec[64:128, :])

            def proj_group(t, d, on_act=False):
                # out^T[d-chunk, tchunk t], K=128 over a head pair per mm;
                # in the drained tail the PSUM->SBUF copy runs on the idle
                # ACT engine instead of DVE
                ts = slice(t * 512, (t + 1) * 512)
                ps = sc_tile()
                for P in range(2):
                    nc.tensor.matmul(
                        ps[:, 0:512],
                        r(wp_sb[:, P * C + d * 128:P * C + (d + 1) * 128]),
                        r(yTp[P][:, ts]),
                        start=(P == 0), stop=(P == 1))
                ob = tail.tile([128, 512], F32, tag="ob", bufs=8, name="ob")
                if on_act:
                    nc.scalar.activation(ob[:], ps[:, 0:512], AF.Copy)
                else:
                    nc.vector.tensor_copy(ob[:], ps[:, 0:512])
                nc.sync.dma_start(outT[d * 128:(d + 1) * 128, ts], ob[:])

            # attn@v lags scores/exp by one l-chunk (software pipeline):
            # the next window's first scores are emitted before the
            # previous window's last attn@v, so neither PE nor ACT idles
            # at window boundaries. lag holds (p, oA, oB, ts, l, et);
            # front holds a pending norm_front result.
            st = {"lag": None, "front": None}

            def emit_attnv(item):
                p_, oA_, oB_, ts_, l_, et_ = item
                nc.tensor.matmul(
                    oA_[0:66, :], var[2 * p_][:, l_, :], et_[:, 0:512],
                    start=(l_ == 0), stop=(l_ == nL - 1))
                nc.tensor.matmul(
                    oB_[0:128, :], var[2 * p_ + 1][:, l_, :],
                    et_[:, 512:1024],
                    start=(l_ == 0), stop=(l_ == nL - 1))
                if l_ == nL - 1:
                    st["front"] = norm_front(p_, oA_, oB_, ts_)

            def attention_t(p, t, extra=None):
                # one 512-wide t-chunk of attention for head pair p
                ts = slice(t * 512, (t + 1) * 512)
                oA = oab_tile("oA")
                oB = oab_tile("oB")
                for l in range(nL):
                    if l == 2 and st["front"] is not None:
                        norm_back(st["front"])
                        st["front"] = None
                    if extra:
                        extra(l)
                    lsl = slice(l * 128, (l + 1) * 128)
                    sc = sc_tile()
                    nc.tensor.matmul(
                        sc[:, 0:512],
                        kcatT[p][0:64, lsl], qnT[p][0:64, ts],
                        start=True, stop=True, tile_position=(0, 0))
                    nc.tensor.matmul(
                        sc[:, 512:1024],
                        kcatT[p][64:128, lsl], qnT[p][64:128, ts],
                        start=True, stop=True, tile_position=(64, 0))
                    if st["lag"] is not None:
                        emit_attnv(st["lag"])
                    et = att2.tile([128, 1024], BF16, tag="exp",
                                   bufs=6, name="et")
                    nc.scalar.activation(et[:], sc[:], AF.Exp,
                                         scale=float(scale))
                    st["lag"] = (p, oA, oB, ts, l, et)

            def load_pair(p):
                # kcat^T kx part: plain DMAs + a Pool-engine add (the DMA
                # accum_op path is SWDGE: ~0.34ns/descriptor, far too slow)
                nc.sync.dma_start(
                    kcatT[p][:, 0:S],
                    kxlT[p * 128:(p + 1) * 128, :])
                ps_ = ph1.tile([128, S], BF16, tag="pos_sb", bufs=1,
                               name="pos_sb")
                nc.sync.dma_start(ps_[:], posT[p * 128:(p + 1) * 128, :])
                nc.gpsimd.tensor_add(kcatT[p][:, 0:S],
                                     kcatT[p][:, 0:S], ps_[:])

            def load_pair_v_dma(p):
                # v_xl is shipped bf16 and DMAs straight into the va dims;
                # ones columns (even: 64..66, odd: 0..64) are memset on the
                # idle Pool engine
                for h in (2 * p, 2 * p + 1):
                    if h % 2 == 0:
                        nc.gpsimd.memset(var[h][:, :, 64:66], 1.0)
                    else:
                        nc.gpsimd.memset(var[h][:, :, 0:64], 1.0)
                    nc.sync.dma_start(
                        var[h][:, 0:nS, VD[h]:VD[h] + HD],
                        vxl.rearrange("(n p) d -> p n d", p=128)
                        [:, :, h * HD:(h + 1) * HD])

            # ---- phase 1: pair-0 loads + projections + attention -----
            with tc.tile_pool(name="ph1", bufs=1) as ph1:
                qTr = qT.rearrange("(n p) t -> p n t", p=128)
                wqr = wqkvT.rearrange("(n p) m -> p n m", p=128)
                # DMA order = criticality for the first attention window
                wqm0 = ph1.tile([128, nC, 128], BF16, tag="wqm0")
                nc.sync.dma_start(wqm0[:], wqr[:, :, 0:128])
                qt = ph1.tile([128, nC, T], BF16, tag="qt")
                nc.sync.dma_start(qt[:, 0:4, 0:512], qTr[:, 0:4, 0:512])
                nc.sync.dma_start(qt[:, 4:8, 0:512], qTr[:, 4:8, 0:512])
                # pair-0 k_xl/pos in 512-col pieces so scores-l0 can start
                # as soon as the first piece lands; v_xl staged between
                # pieces (attn@v l=0 needs it ~8 chunks in)
                pos_sb = ph1.tile([128, S], BF16, tag="pos_sb", bufs=1,
                                  name="pos_sb")

                def kx_piece(c4):
                    cs = slice(c4 * 512, (c4 + 1) * 512)
                    nc.sync.dma_start(kcatT[0][:, cs], kxlT[0:128, cs])
                    nc.sync.dma_start(pos_sb[:, cs], posT[0:128, cs])
                    nc.gpsimd.tensor_add(kcatT[0][:, cs],
                                         kcatT[0][:, cs], pos_sb[:, cs])
                kx_piece(0)
                load_pair_v_dma(0)
                kx_piece(1)
                kx_piece(2)
                wqm2 = ph1.tile([128, nC, 128], BF16, tag="wqm2")
                nc.sync.dma_start(wqm2[:], wqr[:, :, 256:384])
                kx_piece(3)
                wq45 = ph1.tile([128, nC, 2 * 128], BF16, tag="wq45")
                nc.sync.dma_start(wq45[:], wqr[:, :, 512:768])
                nc.sync.dma_start(r(ones_sb[:]), r(ones_in[:, 0:64]))
                nc.sync.dma_start(r(sel_sb[:]), r(ones_in[:, 64:192]))
                for t in (1, 2, 3):
                    ts = slice(t * 512, (t + 1) * 512)
                    nc.sync.dma_start(qt[:, :, ts], qTr[:, :, ts])

                qkv_m_t(wqm0, 0, 0)   # qn^T pair0 t0 — unblocks attention

                # t0: va casts first, then kn-pair0 (m2) and vn interleaved
                # at the pace the DMAs land, rest of m0 at the end
                t0_sched = {8: lambda: qkv_m_t(wqm2, 2, 0),
                            10: lambda: qkv_m_t(wqm2, 2, 1),
                            12: lambda: qkv_m_t(wqm2, 2, 2),
                            14: lambda: qkv_m_t(wqm2, 2, 3),
                            28: lambda: qkv_m_t(wqm0, 0, 1),
                            29: lambda: qkv_m_t(wqm0, 0, 2),
                            31: lambda: qkv_m_t(wqm0, 0, 3)}
                for i in range(nT):
                    li = (9, 11, 13, 15)[i] if i < 4 else 16 + (i - 4)
                    t0_sched[li] = (lambda i=i: vn_chunk(i))

                def mk_extra(sched):
                    return lambda l: sched.get(l, lambda: None)()

                attention_t(0, 0, mk_extra(t0_sched))

                # m1/m3 (pair-1 qn/kn) spread over the t1..t3 windows,
                # ~3 lumps per window to stay inside ACT's slack
                wqm13 = ph1.tile([128, nC, 128], BF16, tag="wqm13",
                                 bufs=2, name="wqm13_1")
                nc.sync.dma_start(wqm13[:], wqr[:, :, 128:256])
                t1_sched = {8: lambda: qkv_m_t(wqm13, 1, 0),
                            15: lambda: qkv_m_t(wqm13, 1, 1),
                            22: lambda: qkv_m_t(wqm13, 1, 2)}
                attention_t(0, 1, mk_extra(t1_sched))

                wqm3 = ph1.tile([128, nC, 128], BF16, tag="wqm13",
                                bufs=2, name="wqm13_3")
                nc.sync.dma_start(wqm3[:], wqr[:, :, 384:512])
                t2_sched = {8: lambda: qkv_m_t(wqm13, 1, 3),
                            15: lambda: qkv_m_t(wqm3, 3, 0),
                            22: lambda: qkv_m_t(wqm3, 3, 1)}
                attention_t(0, 2, mk_extra(t2_sched))

                load_pair(1)
                load_pair_v_dma(1)
                t3_sched = {8: lambda: qkv_m_t(wqm3, 3, 2),
                            15: lambda: qkv_m_t(wqm3, 3, 3)}
                attention_t(0, 3, mk_extra(t3_sched))

            # ---- pair-1 attention with drip-fed output projection ----
            with tc.tile_pool(name="tail_sb", bufs=1) as tail:
                wp_sb = tail.tile([128, 2 * C], F32, tag="wp")
                nc.sync.dma_start(r(wp_sb[:]), r(wp2[:]))

                attention_t(1, 0, None)
                for t in range(1, nTc):
                    s = {}
                    for d in range(nC):
                        s[4 + 3 * d] = (lambda tt=t - 1, d=d:
                                        proj_group(tt, d))
                    attention_t(1, t, mk_extra(s))
                # drain the pipeline: last attn@v chunk, final normalize,
                # final projection (ACT does the copies — it is idle here)
                emit_attnv(st["lag"])
                norm_back(st["front"])
                for d in range(nC):
                    proj_group(nTc - 1, d, on_act=True)
                if timing_mode:
                    tt_ = tail.tile([128, 128], F32, tag="tiny")
                    nc.sync.dma_start(tt_[:], dummy[:])
                    nc.sync.dma_start(tiny[:], tt_[:])

    nc.compile()
    return nc


_cache = {}


def _program(T, S, C):
    key = (T, S, C)
    if key not in _cache:
        _cache[key] = build_program(T, S, C)
    return _cache[key]


def _ones_sel():
    # cols 0:64 all-ones; cols 64:192 the bc selector: row 64 broadcasts
    # A-den to out parts 0..63, row 65 broadcasts B-den to 64..127
    arr = np.zeros((128, 192), np.float32)
    arr[:, 0:64] = 1.0
    arr[0, 64:192] = 1.0
    return arr


def core_inputs(q, k_xl, v_xl, W_qkv, W_proj, pos_emb, core):
    """Host-side shard prep for one core (slicing + layout transposes)."""
    C = q.shape[2]
    b = core // 4
    h0 = (core % 4) * HPC
    cols = slice(h0 * HD, (h0 + HPC) * HD)
    rows = np.r_[h0 * HD:(h0 + HPC) * HD]
    wrows = np.concatenate([rows, C + rows, 2 * C + rows])
    # wp2: [128, 2C]; pair P block = W_proj[:, pair-P channel cols].T
    wp2 = np.concatenate(
        [W_proj[:, (h0 + 2 * P) * HD:(h0 + 2 * P + 2) * HD].T
         for P in range(2)], axis=1)
    return {
        "qT": np.ascontiguousarray(q[b].T).astype(np_bf16),
        "wqkvT": np.ascontiguousarray(W_qkv[wrows].T).astype(np_bf16),
        "wp2": np.ascontiguousarray(wp2),
        "kxlT": np.ascontiguousarray(k_xl[b].T[cols]).astype(np_bf16),
        "posT": np.ascontiguousarray(pos_emb.T[cols]).astype(np_bf16),
        "vxl": np.ascontiguousarray(v_xl[b][:, cols]).astype(np_bf16),
        "ones": _ones_sel(),
    }


def kernel(q, k_xl, v_xl, W_qkv, W_proj, pos_emb, is_causal):
    q = np.asarray(q, np.float32)
    k_xl = np.asarray(k_xl, np.float32)
    v_xl = np.asarray(v_xl, np.float32)
    W_qkv = np.asarray(W_qkv, np.float32)
    W_proj = np.asarray(W_proj, np.float32)
    pos_emb = np.asarray(pos_emb, np.float32)
    B, T, C = q.shape
    S = k_xl.shape[1]

    nc = _program(T, S, C)
    in_maps = [core_inputs(q, k_xl, v_xl, W_qkv, W_proj, pos_emb, c)
               for c in range(N_CORES)]
    res = run_bass_kernel_spmd(nc, in_maps, list(range(N_CORES)))

    out = np.zeros((B, T, C), np.float32)
    for c in range(N_CORES):
        out[c // 4] += res.results[c]["outT"].T
    return out
